# revision 33
# baseline (speedup 1.0000x reference)
"""GQA kernel for Trainium2, tensor-parallel over 8 NeuronCores.

Problem: X[2,2048,4096] -> GQA(H=32 heads, G=8 kv-groups, D=128) -> out[2,2048,4096].
Sharding: core c owns query heads 4c..4c+3 (512 q-features) and KV group c
(128 kv-features), plus the matching Wo row-slice. Each core computes a full
[4096,4096] partial of the output projection; the host sums the 8 partials
(bf16) and adds bo.

Device layouts (feature-on-partition so no activations need transposing
except V, which gets 32 PE transposes):
  Q_T [128d, 4h, 4096tok]  K_T [128d, 4096tok]  V [128tok, 32tt, 128d]
  scores_T[k,q] = K_tile.T @ Q_tile    (d contracts on partitions)
  Score PSUM comes in 2-bank pairs so ONE exp covers [128,1024] (halves the
  scalar-engine instruction count). Softmax denominator: bf16 DVE tree over
  the 16 kt tiles, then ONE ones-matrix matmul sums the 128 k-partitions AND
  broadcasts to all 128 output partitions; 1/den is computed as Exp(-Ln(den))
  on the scalar engine (both functions live in the same ACT table set, and
  the DVE reciprocal instruction is ~6.5ns/elem — far slower).
  mid_T[d,q] = sum_kt V_tile.T @ P_tile, scaled by the broadcast 1/denom
  out[tok,hid] = sum_h mid_T_h.T @ Wo_h  (accumulated over the 4 heads in PSUM)

Schedule: phase A (QKV proj) runs 16 token-tiles at ~100% PE, with K/V
projections first on tile 0 so compute starts as soon as the small wk DMA
lands (xt streams on the scalar-engine DMA queue in parallel with weights on
the sync queue). Phase B runs 32 softmax steps (b, q4, h) in a 3-deep
software pipeline with the phase-C output-projection n-groups interleaved
into every step as PE filler.
"""

import os

import ml_dtypes
import numpy as np

import concourse.bass as bass
import concourse.mybir as mybir
import concourse.tile as tile
from concourse.bass_utils import run_bass_kernel_spmd
from concourse.masks import make_identity

B, S, HID = 2, 2048, 4096
H, G, D = 32, 8, 128
T = B * S            # 4096 tokens
NCORES = 8
HPC = H // NCORES    # 4 heads per core
FPC = HPC * D        # 512 q-features per core
TT = 256             # phase-A token tile
NTT = T // TT        # 16
SCALE = 1.0 / float(np.sqrt(D))

BF16 = mybir.dt.bfloat16
F32 = mybir.dt.float32
npbf16 = ml_dtypes.bfloat16

LAST_RESULT = None  # test.py reads exec_time_ns / trace path from here


def _legalize_single_wait(nc):
    """This container's walrus accepts at most ONE sync-wait per instruction;
    Tile emits several. Split surplus waits onto EVSEM carrier instructions
    inserted just before the offender on the same engine."""
    n_carrier = 0
    for f in nc.m.functions:
        for blk in f.blocks:
            insts = blk.instructions
            out = []
            changed = False
            for ins in insts:
                si = ins.sync_info
                if si is not None and len(si.on_wait) > 1:
                    waits = list(si.on_wait)
                    ups = list(si.on_update)
                    for w in waits[:-1]:
                        n_carrier += 1
                        ev = mybir.InstEventSemaphore(
                            name=f"I-waitfix-{n_carrier}")
                        ev.engine = ins.engine
                        ev.sync_info = mybir.SyncInfo(on_wait=[w], on_update=[])
                        out.append(ev)
                    ins.sync_info = mybir.SyncInfo(
                        on_wait=[waits[-1]], on_update=ups)
                    changed = True
                out.append(ins)
            if changed:
                blk.instructions = out
    return n_carrier


def _build_program():
    nc = bass.Bass()

    xt = nc.dram_tensor("xt", [128, NTT, 32, TT], BF16, kind="ExternalInput")
    # wq is f-major: [128, HPC, 32kt, 128] so one Q feature-group's weights
    # are a single contiguous 1MB DMA with 8KB partition rows
    wq = nc.dram_tensor("wq", [128, HPC, 32, D], BF16, kind="ExternalInput")
    wk = nc.dram_tensor("wk", [128, 32, D], BF16, kind="ExternalInput")
    wv = nc.dram_tensor("wv", [128, 32, D], BF16, kind="ExternalInput")
    wo = nc.dram_tensor("wo", [128, HPC, HID], BF16, kind="ExternalInput")
    # bq (HPC cols) | bk | bv packed in one tensor -> one small DMA
    ball = nc.dram_tensor("ball", [128, HPC + 2], F32, kind="ExternalInput")
    out = nc.dram_tensor("out", [T, HID], BF16, kind="ExternalOutput")

    Id = mybir.ActivationFunctionType.Identity
    Exp = mybir.ActivationFunctionType.Exp
    Ln = mybir.ActivationFunctionType.Ln

    with nc.allow_low_precision("bf16 softmax denominator / scale factors"), \
         tile.TileContext(nc) as tc:
        with (
            tc.tile_pool(name="persist", bufs=1) as persist,
            tc.tile_pool(name="stream", bufs=4) as stream,
            tc.tile_pool(name="tree", bufs=1) as treep,
            tc.tile_pool(name="accp", bufs=2) as accp,
            tc.tile_pool(name="ldenp", bufs=2) as ldenp,
            tc.tile_pool(name="rbp", bufs=2) as rbp,
            tc.tile_pool(name="midp", bufs=14) as midp,
            tc.tile_pool(name="outp", bufs=4) as outp,
            tc.tile_pool(name="ppair", bufs=2, space="PSUM") as ppair,
            tc.tile_pool(name="pm", bufs=2, space="PSUM") as pm,
            tc.tile_pool(name="pc", bufs=2, space="PSUM") as pc,
        ):
            # ---- persistent SBUF tensors ----
            wq_sb = persist.tile([128, HPC, 32, D], BF16, tag="wbig")
            wk_sb = persist.tile([128, 32, D], BF16, tag="wk")
            wv_sb = persist.tile([128, 32, D], BF16, tag="wv")
            q_sb = persist.tile([128, HPC, T], BF16, tag="q")
            k_sb = persist.tile([128, T], BF16, tag="k")
            v_sb = persist.tile([128, 32, D], BF16, tag="v")
            ball_sb = persist.tile([128, HPC + 2], F32, tag="ball")
            ident = persist.tile([128, 128], BF16, tag="ident")
            ones_mat = persist.tile([128, 128], BF16, tag="onesm")
            bk_ap = ball_sb[:, HPC:HPC + 1]
            bv_ap = ball_sb[:, HPC + 1:HPC + 2]

            # Startup DMAs. Weights go on the Sync HWDGE queue; xt token
            # tiles stream on the Scalar HWDGE queue so the two transfer in
            # parallel. Tile 0's group order is K, Q0, V, Q1..Q3 to match
            # the DMA arrival order below.
            xt0 = stream.tile([128, 32, TT], BF16, tag="s16k", name="xt0")
            for cch in range(4):
                nc.sync.dma_start(out=wk_sb[:, cch * 8:(cch + 1) * 8, :],
                                  in_=wk[:, cch * 8:(cch + 1) * 8, :])
                nc.scalar.dma_start(out=xt0[:, cch * 8:(cch + 1) * 8, :],
                                    in_=xt[:, 0, cch * 8:(cch + 1) * 8, :])
            nc.sync.dma_start(out=wq_sb[:, 0, :, :], in_=wq[:, 0, :, :])
            nc.sync.dma_start(out=ball_sb, in_=ball[:, :])
            nc.sync.dma_start(out=wv_sb, in_=wv[:, :, :])
            for f in range(1, HPC):
                nc.sync.dma_start(out=wq_sb[:, f, :, :], in_=wq[:, f, :, :])
            make_identity(nc, ident)
            nc.vector.memset(ones_mat, 1.0)

            # PE pre-warm: run throwaway matmuls while the first weight DMAs
            # land, so the tensor engine's p-state clock is fully ramped when
            # real work starts. Output is drained to a scratch tile, unused.
            warm_ps = pc.tile([128, 128], F32, tag="cgrp", name="warmps")
            for w in range(24):
                nc.tensor.matmul(warm_ps, lhsT=ones_mat, rhs=ones_mat,
                                 start=(w == 0), stop=(w == 23))
            warm_sb = accp.tile([128, 128], BF16, tag="vstage", name="warmsb")
            nc.vector.tensor_copy(out=warm_sb, in_=warm_ps)

            # =========== Phase A: QKV projections ===========
            def emit_A_group(tt, g, xt_t):
                """One of the six matmul groups of A-tile tt: g 0..3 = q-head
                features, 4 = k, 5 = v (+ V transposes)."""
                t0 = tt * TT
                with nc.named_scope("phaseA"):
                    if g < HPC:
                        f = g
                        ps = pm.tile([128, TT], F32, tag="psm",
                                     name=f"psq{tt}_{f}")
                        for kt in range(32):
                            nc.tensor.matmul(
                                ps, lhsT=wq_sb[:, f, kt, :],
                                rhs=xt_t[:, kt, :],
                                start=(kt == 0), stop=(kt == 31))
                        nc.scalar.activation(
                            out=q_sb[:, f, t0:t0 + TT], in_=ps, func=Id,
                            bias=ball_sb[:, f:f + 1], scale=1.0)
                    elif g == HPC:
                        ps = pm.tile([128, TT], F32, tag="psm",
                                     name=f"psk{tt}")
                        for kt in range(32):
                            nc.tensor.matmul(ps, lhsT=wk_sb[:, kt, :],
                                             rhs=xt_t[:, kt, :],
                                             start=(kt == 0), stop=(kt == 31))
                        nc.scalar.activation(
                            out=k_sb[:, t0:t0 + TT], in_=ps,
                            func=Id, bias=bk_ap, scale=1.0)
                    else:
                        ps = pm.tile([128, TT], F32, tag="psm",
                                     name=f"psv{tt}")
                        for kt in range(32):
                            nc.tensor.matmul(ps, lhsT=wv_sb[:, kt, :],
                                             rhs=xt_t[:, kt, :],
                                             start=(kt == 0), stop=(kt == 31))
                        vstage = accp.tile([128, TT], BF16, tag="vstage",
                                           name=f"vs{tt}")
                        nc.scalar.activation(out=vstage, in_=ps, func=Id,
                                             bias=bv_ap, scale=1.0)
                        for j in range(TT // 128):
                            trp = pc.tile([128, 128], BF16, tag="cgrp",
                                          name=f"tr{tt}_{j}")
                            nc.tensor.transpose(
                                trp, vstage[:, j * 128:(j + 1) * 128], ident)
                            nc.vector.tensor_copy(
                                out=v_sb[:, tt * (TT // 128) + j, :], in_=trp)

            # `interleave`: a list of thunks (warmup attention score+exp
            # emissions) sprinkled between the six matmul groups of this tile
            def emit_A(tt, interleave=()):
                il = list(interleave)
                ipos = [0]

                def sprinkle(k):
                    for _ in range(k):
                        if ipos[0] < len(il):
                            il[ipos[0]]()
                            ipos[0] += 1

                if tt == 0:
                    xt_t = xt0
                else:
                    xt_t = stream.tile([128, 32, TT], BF16, tag="s16k",
                                       name=f"xt{tt}")
                    nc.scalar.dma_start(out=xt_t, in_=xt[:, tt, :, :])
                # Tile 0 matches the DMA arrival order: wk, wq_f0, wv, wq_f1+
                order = (4, 0, 5, 1, 2, 3) if tt == 0 else (0, 1, 2, 3, 4, 5)
                for g in order:
                    emit_A_group(tt, g, xt_t)
                    if g < HPC:
                        sprinkle(3)
                    elif g == HPC:
                        sprinkle(2)
                    else:
                        sprinkle(len(il))

            # =========== Phase B: attention, software-pipelined ===========
            # step key = (b, q4, h); 32 steps.
            state = {}

            def open_step(key):
                b, q4, h = key
                state[key] = {"p": stream.tile(
                    [128, 16, 512], BF16, tag="s16k",
                    name=f"p{b}_{q4}_{h}")}

            def emit_S1_mm(key, kt):
                """Score matmul for kt; on odd kt also the pair exp."""
                b, q4, h = key
                st = state[key]
                with nc.named_scope("phaseB"):
                    koff = b * S + kt * 128
                    qoff = b * S + q4 * 512
                    if kt % 2 == 0:
                        st["pair"] = ppair.tile(
                            [128, 2, 512], F32, tag="pair",
                            name=f"pss{b}_{q4}_{h}_{kt}")
                    nc.tensor.matmul(st["pair"][:, kt % 2, :],
                                     lhsT=k_sb[:, koff:koff + 128],
                                     rhs=q_sb[:, h, qoff:qoff + 512],
                                     start=True, stop=True)
                    if kt % 2 == 1:
                        nc.scalar.activation(
                            out=st["p"][:, kt - 1:kt + 1, :],
                            in_=st["pair"], func=Exp, scale=SCALE)

            def emit_S2(key):
                b, q4, h = key
                st = state[key]
                p_t = st["p"]
                with nc.named_scope("phaseB"):
                    t8 = treep.tile([128, 8, 512], BF16, tag="t8",
                                    name=f"t8_{b}_{q4}_{h}")
                    nc.vector.tensor_add(t8, p_t[:, 0:8, :], p_t[:, 8:16, :])
                    t4 = treep.tile([128, 4, 512], BF16, tag="t4",
                                    name=f"t4_{b}_{q4}_{h}")
                    nc.vector.tensor_add(t4, t8[:, 0:4, :], t8[:, 4:8, :])
                    t2 = treep.tile([128, 2, 512], BF16, tag="t2",
                                    name=f"t2_{b}_{q4}_{h}")
                    nc.vector.tensor_add(t2, t4[:, 0:2, :], t4[:, 2:4, :])
                    acc = accp.tile([128, 512], BF16, tag="acc",
                                    name=f"acc{b}_{q4}_{h}")
                    nc.vector.tensor_add(acc, t2[:, 0, :], t2[:, 1, :])
                    st["acc"] = acc

            def emit_S3(key):
                # One ones-matrix matmul sums acc over the 128 k-partitions
                # and broadcasts the result to all 128 partitions; then
                # 1/den = Exp(-Ln(den)) on the scalar engine (same ACT table
                # set as the softmax Exp, so no table reloads).
                b, q4, h = key
                st = state[key]
                with nc.named_scope("phaseB"):
                    den_ps = pc.tile([128, 512], F32, tag="cgrp",
                                     name=f"dn{b}_{q4}_{h}")
                    nc.tensor.matmul(den_ps, lhsT=ones_mat, rhs=st["acc"],
                                     start=True, stop=True)
                    lden = ldenp.tile([128, 512], F32, tag="lden",
                                      name=f"ld{b}_{q4}_{h}")
                    nc.scalar.activation(out=lden, in_=den_ps, func=Ln)
                    rb = rbp.tile([128, 512], BF16, tag="rb",
                                  name=f"rb{b}_{q4}_{h}")
                    nc.scalar.activation(out=rb, in_=lden, func=Exp,
                                         scale=-1.0)
                    st["rb"] = rb

            def emit_S4_av(key, kt0, kt1):
                b, q4, h = key
                st = state[key]
                with nc.named_scope("phaseB"):
                    if kt0 == 0:
                        st["psm"] = pm.tile([128, 512], F32, tag="psm",
                                            name=f"psm{b}_{q4}_{h}")
                    for kt in range(kt0, kt1):
                        nc.tensor.matmul(st["psm"],
                                         lhsT=v_sb[:, b * 16 + kt, :],
                                         rhs=st["p"][:, kt, :],
                                         start=(kt == 0), stop=(kt == 15))

            def emit_S4_mul(key):
                b, q4, h = key
                st = state[key]
                with nc.named_scope("phaseB"):
                    mid = midp.tile([128, 512], BF16, tag="mid",
                                    name=f"mid{b}_{q4}_{h}")
                    nc.vector.tensor_mul(out=mid, in0=st["psm"], in1=st["rb"])
                    st["mid"] = mid

            # =========== Phase C: one output n-group ===========
            # group = (tt32, n): out tokens [tt32*128,+128), cols [n*512,+512)
            ndrain = [0]

            def emit_C_group(tt32, n, drain, pool="pc", dma_eng="sync"):
                b = tt32 // 16
                q4 = (tt32 % 16) // 4
                j = tt32 % 4
                with nc.named_scope("phaseC"):
                    if pool == "pm":
                        pso = pm.tile([128, 512], F32, tag="psm",
                                      name=f"pso{tt32}_{n}")
                    elif pool == "pair":
                        pso = ppair.tile([128, 2, 512], F32, tag="pair",
                                         name=f"pso{tt32}_{n}")[:, 0, :]
                    else:
                        pso = pc.tile([128, 512], F32, tag="cgrp",
                                      name=f"pso{tt32}_{n}")
                    for h in range(HPC):
                        mid = state[(b, q4, h)]["mid"]
                        nc.tensor.matmul(
                            pso, lhsT=mid[:, j * 128:(j + 1) * 128],
                            rhs=wo_sb[:, h, n * 512:(n + 1) * 512],
                            start=(h == 0), stop=(h == HPC - 1))
                    ot = outp.tile([128, 512], BF16, tag="outstage",
                                   name=f"ot{tt32}_{n}")
                    # ACT carries the pair-exps + Ln/Exp chain; give it only
                    # 1 in 4 of the C drains, the rest go to the DVE
                    if drain == "alt":
                        drain = "s" if ndrain[0] % 4 == 3 else "v"
                    elif drain == "alt2":
                        drain = "s" if ndrain[0] % 2 == 1 else "v"
                    if drain == "v":
                        nc.vector.tensor_copy(out=ot, in_=pso)
                    else:
                        nc.scalar.copy(out=ot, in_=pso)
                    ndrain[0] += 1
                    eng = nc.sync if dma_eng == "sync" else nc.scalar
                    eng.dma_start(
                        out=out[tt32 * 128:(tt32 + 1) * 128,
                                n * 512:(n + 1) * 512],
                        in_=ot)

            # ---------------- program order ----------------
            steps = [(b, q4, h)
                     for b in range(B) for q4 in range(4) for h in range(HPC)]
            nsteps = len(steps)

            # A tiles 0..13 plain; warmup S1+exp of B-steps 0/1 interleaved
            # into tiles 14/15 (their q/k inputs are complete after tile 13)
            for tt in range(NTT - 2):
                emit_A(tt)
            for w, tt in ((0, NTT - 2), (1, NTT - 1)):
                open_step(steps[w])
                emit_A(tt, interleave=[
                    (lambda key=steps[w], kt=kt: emit_S1_mm(key, kt))
                    for kt in range(16)])
            emit_S2(steps[0])

            # wo shares the wq slot; load once phase A's last wq read retires
            wo_sb = persist.tile([128, HPC, HID], BF16, tag="wbig")
            nc.sync.dma_start(out=wo_sb, in_=wo[:, :, :])

            # C n-group queue: groups for q4-chunk become ready once all 4
            # heads' muls are emitted (mul for step s lands in slot s+2).
            cqueue = []
            cq_pos = [0]

            def push_ready_groups(slot):
                # step index s completes its mul in slot s+2; a q4 chunk
                # (steps 4k..4k+3) unlocks its 4 token-tiles after slot 4k+5
                while True:
                    k = len(cqueue) // 32  # q4 chunks fully pushed so far
                    if k >= 8 or 4 * k + 5 > slot:
                        break
                    b, q4 = steps[4 * k][0], steps[4 * k][1]
                    for j in range(4):
                        tt32 = b * 16 + q4 * 4 + j
                        for n in range(8):
                            cqueue.append((tt32, n))

            def emit_C_quota(quota, drain="alt"):
                e = 0
                while e < quota and cq_pos[0] < len(cqueue):
                    emit_C_group(*cqueue[cq_pos[0]], drain=drain)
                    cq_pos[0] += 1
                    e += 1

            # main slot loop; slots 0/1 were absorbed into the A-tail prologue
            for i in range(2, nsteps + 2):
                s1 = steps[i] if i < nsteps else None
                s2 = steps[i - 1] if 1 <= i - 1 < nsteps else None
                s34 = steps[i - 2] if 2 <= i else None

                if s1 is not None:
                    open_step(s1)
                if s34 is not None:
                    # first AV quarter up front: psm allocates while its ring
                    # is empty, so it never stalls behind a full ring
                    emit_S4_av(s34, 0, 4)
                if s2 is not None:
                    emit_S2(s2)
                # interleave: 4-score bursts, AV quarters, C groups; the
                # denominator chain (S3) goes at kt==2 so the scalar engine
                # starts the slot with the first pair exp, not with Ln
                for kt in range(16):
                    if s1 is not None:
                        emit_S1_mm(s1, kt)
                    if kt == 2 and s34 is not None:
                        emit_S3(s34)
                    if kt % 4 == 1 and kt > 1 and s34 is not None:
                        emit_S4_av(s34, kt - 1, kt + 3)
                    if kt % 4 == 3:
                        emit_C_quota(2)
                if s34 is not None:
                    emit_S4_mul(s34)
                push_ready_groups(i)
                # top-up C toward a uniform drain rate over the slot budget.
                # In the last slots the scalar engine is winding down its
                # exps, so split drains 50/50 to keep the DVE from backing up
                # (a late S4_mul stalls the next psm allocation on the PE).
                want = (len(cqueue) * (i + 1)) // (nsteps + 2)
                cap = 16 if i >= nsteps - 4 else 12
                dr = "alt2" if i >= nsteps - 6 else "alt"
                emit_C_quota(max(0, min(want - cq_pos[0], cap)), drain=dr)

            # flush remaining output groups: the pair/psm PSUM rings are idle
            # now, so cycle all three pools (6 banks) and split drains evenly
            # between the scalar and vector engines to avoid ring stalls
            push_ready_groups(1000)
            fl = 0
            while cq_pos[0] < len(cqueue):
                emit_C_group(*cqueue[cq_pos[0]],
                             drain=("v" if fl % 2 == 0 else "s"),
                             pool=("pc", "pm", "pair")[fl % 3])
                cq_pos[0] += 1
                fl += 1

    return nc


_cached_nc = None


def _get_program():
    global _cached_nc
    if _cached_nc is None:
        _cached_nc = _build_program()
        _legalize_single_wait(_cached_nc)
    return _cached_nc


def kernel(X, Wq, bq, Wk, bk, Wv, bv, Wo, bo):
    global LAST_RESULT
    X = np.asarray(X, np.float32)
    Wq = np.asarray(Wq, np.float32)
    Wk = np.asarray(Wk, np.float32)
    Wv = np.asarray(Wv, np.float32)
    Wo = np.asarray(Wo, np.float32)
    bq = np.asarray(bq, np.float32)
    bk = np.asarray(bk, np.float32)
    bv = np.asarray(bv, np.float32)
    bo = np.asarray(bo, np.float32)

    XT = np.ascontiguousarray(X.reshape(T, HID).T)          # [HID, T]
    xt_host = np.ascontiguousarray(
        XT.reshape(32, 128, NTT, TT).transpose(1, 2, 0, 3)).astype(npbf16)

    in_maps = []
    for c in range(NCORES):
        wq_c = Wq[:, c * FPC:(c + 1) * FPC]
        wk_c = Wk[:, c * D:(c + 1) * D]
        wv_c = Wv[:, c * D:(c + 1) * D]
        wo_c = Wo[c * FPC:(c + 1) * FPC, :]
        ball_c = np.concatenate([
            bq[c * FPC:(c + 1) * FPC].reshape(HPC, 128).T,
            bk[c * D:(c + 1) * D].reshape(D, 1),
            bv[c * D:(c + 1) * D].reshape(D, 1),
        ], axis=1)
        in_maps.append({
            "xt": xt_host,
            "wq": np.ascontiguousarray(
                wq_c.reshape(32, 128, HPC, D).transpose(1, 2, 0, 3),
                ).astype(npbf16),
            "wk": np.ascontiguousarray(
                wk_c.reshape(32, 128, D).transpose(1, 0, 2)).astype(npbf16),
            "wv": np.ascontiguousarray(
                wv_c.reshape(32, 128, D).transpose(1, 0, 2)).astype(npbf16),
            "wo": np.ascontiguousarray(
                wo_c.reshape(HPC, 128, HID).transpose(1, 0, 2)).astype(npbf16),
            "ball": np.ascontiguousarray(ball_c).astype(np.float32),
        })

    nc = _get_program()
    res = run_bass_kernel_spmd(
        nc, in_maps, list(range(NCORES)),
        tmpdir=os.environ.get("BASS_TMPDIR") or None)
    LAST_RESULT = res

    acc = np.zeros((T, HID), np.float32)
    for c in range(NCORES):
        acc += np.asarray(res.results[c]["out"], np.float32)
    return (acc + bo[None, :]).reshape(B, S, HID).astype(np.float32)


# revision 34
# speedup vs baseline: 1.0059x; 1.0059x over previous
"""GQA kernel for Trainium2, tensor-parallel over 8 NeuronCores.

Problem: X[2,2048,4096] -> GQA(H=32 heads, G=8 kv-groups, D=128) -> out[2,2048,4096].
Sharding: core c owns query heads 4c..4c+3 (512 q-features) and KV group c
(128 kv-features), plus the matching Wo row-slice. Each core computes a full
[4096,4096] partial of the output projection; the host sums the 8 partials
(bf16) and adds bo.

Device layouts (feature-on-partition so no activations need transposing
except V, which gets 32 PE transposes):
  Q_T [128d, 4h, 4096tok]  K_T [128d, 4096tok]  V [128tok, 32tt, 128d]
  scores_T[k,q] = K_tile.T @ Q_tile    (d contracts on partitions)
  Score PSUM comes in 2-bank pairs so ONE exp covers [128,1024] (halves the
  scalar-engine instruction count). Softmax denominator: bf16 DVE tree over
  the 16 kt tiles, then ONE ones-matrix matmul sums the 128 k-partitions AND
  broadcasts to all 128 output partitions; 1/den is computed as Exp(-Ln(den))
  on the scalar engine (both functions live in the same ACT table set, and
  the DVE reciprocal instruction is ~6.5ns/elem — far slower).
  mid_T[d,q] = sum_kt V_tile.T @ P_tile, scaled by the broadcast 1/denom
  out[tok,hid] = sum_h mid_T_h.T @ Wo_h  (accumulated over the 4 heads in PSUM)

Schedule: phase A (QKV proj) runs 16 token-tiles at ~100% PE, with K/V
projections first on tile 0 so compute starts as soon as the small wk DMA
lands (xt streams on the scalar-engine DMA queue in parallel with weights on
the sync queue). Phase B runs 32 softmax steps (b, q4, h) in a 3-deep
software pipeline with the phase-C output-projection n-groups interleaved
into every step as PE filler.
"""

import os

import ml_dtypes
import numpy as np

import concourse.bass as bass
import concourse.mybir as mybir
import concourse.tile as tile
from concourse.bass_utils import run_bass_kernel_spmd
from concourse.masks import make_identity

B, S, HID = 2, 2048, 4096
H, G, D = 32, 8, 128
T = B * S            # 4096 tokens
NCORES = 8
HPC = H // NCORES    # 4 heads per core
FPC = HPC * D        # 512 q-features per core
TT = 256             # phase-A token tile
NTT = T // TT        # 16
SCALE = 1.0 / float(np.sqrt(D))

BF16 = mybir.dt.bfloat16
F32 = mybir.dt.float32
npbf16 = ml_dtypes.bfloat16

LAST_RESULT = None  # test.py reads exec_time_ns / trace path from here


def _legalize_single_wait(nc):
    """This container's walrus accepts at most ONE sync-wait per instruction;
    Tile emits several. Split surplus waits onto EVSEM carrier instructions
    inserted just before the offender on the same engine."""
    n_carrier = 0
    for f in nc.m.functions:
        for blk in f.blocks:
            insts = blk.instructions
            out = []
            changed = False
            for ins in insts:
                si = ins.sync_info
                if si is not None and len(si.on_wait) > 1:
                    waits = list(si.on_wait)
                    ups = list(si.on_update)
                    for w in waits[:-1]:
                        n_carrier += 1
                        ev = mybir.InstEventSemaphore(
                            name=f"I-waitfix-{n_carrier}")
                        ev.engine = ins.engine
                        ev.sync_info = mybir.SyncInfo(on_wait=[w], on_update=[])
                        out.append(ev)
                    ins.sync_info = mybir.SyncInfo(
                        on_wait=[waits[-1]], on_update=ups)
                    changed = True
                out.append(ins)
            if changed:
                blk.instructions = out
    return n_carrier


def _build_program():
    nc = bass.Bass()

    xt = nc.dram_tensor("xt", [128, NTT, 32, TT], BF16, kind="ExternalInput")
    # wq is f-major: [128, HPC, 32kt, 128] so one Q feature-group's weights
    # are a single contiguous 1MB DMA with 8KB partition rows
    wq = nc.dram_tensor("wq", [128, HPC, 32, D], BF16, kind="ExternalInput")
    wk = nc.dram_tensor("wk", [128, 32, D], BF16, kind="ExternalInput")
    wv = nc.dram_tensor("wv", [128, 32, D], BF16, kind="ExternalInput")
    wo = nc.dram_tensor("wo", [128, HPC, HID], BF16, kind="ExternalInput")
    # bq (HPC cols) | bk | bv packed in one tensor -> one small DMA
    ball = nc.dram_tensor("ball", [128, HPC + 2], F32, kind="ExternalInput")
    out = nc.dram_tensor("out", [T, HID], BF16, kind="ExternalOutput")

    Id = mybir.ActivationFunctionType.Identity
    Exp = mybir.ActivationFunctionType.Exp
    Ln = mybir.ActivationFunctionType.Ln

    with nc.allow_low_precision("bf16 softmax denominator / scale factors"), \
         tile.TileContext(nc) as tc:
        with (
            tc.tile_pool(name="persist", bufs=1) as persist,
            tc.tile_pool(name="stream", bufs=4) as stream,
            tc.tile_pool(name="tree", bufs=1) as treep,
            tc.tile_pool(name="accp", bufs=2) as accp,
            tc.tile_pool(name="ldenp", bufs=2) as ldenp,
            tc.tile_pool(name="rbp", bufs=2) as rbp,
            tc.tile_pool(name="midp", bufs=14) as midp,
            tc.tile_pool(name="outp", bufs=4) as outp,
            tc.tile_pool(name="ppair", bufs=2, space="PSUM") as ppair,
            tc.tile_pool(name="pm", bufs=2, space="PSUM") as pm,
            tc.tile_pool(name="pc", bufs=2, space="PSUM") as pc,
        ):
            # ---- persistent SBUF tensors ----
            wq_sb = persist.tile([128, HPC, 32, D], BF16, tag="wbig")
            wk_sb = persist.tile([128, 32, D], BF16, tag="wk")
            wv_sb = persist.tile([128, 32, D], BF16, tag="wv")
            q_sb = persist.tile([128, HPC, T], BF16, tag="q")
            k_sb = persist.tile([128, T], BF16, tag="k")
            v_sb = persist.tile([128, 32, D], BF16, tag="v")
            ball_sb = persist.tile([128, HPC + 2], F32, tag="ball")
            ident = persist.tile([128, 128], BF16, tag="ident")
            ones_mat = persist.tile([128, 128], BF16, tag="onesm")
            bk_ap = ball_sb[:, HPC:HPC + 1]
            bv_ap = ball_sb[:, HPC + 1:HPC + 2]

            # Startup DMAs. Weights go on the Sync HWDGE queue; xt token
            # tiles stream on the Scalar HWDGE queue so the two transfer in
            # parallel. Tile 0's group order is K, Q0, V, Q1..Q3 to match
            # the DMA arrival order below.
            # The K->Q0 startup chain rides the Sync queue: xt0's first half
            # and wq_f0. The small wk chunks ride the Scalar queue alone, so
            # neither queue blocks the other's critical bytes.
            xt0 = stream.tile([128, 32, TT], BF16, tag="s16k", name="xt0")
            for cch in range(4):
                nc.scalar.dma_start(out=wk_sb[:, cch * 8:(cch + 1) * 8, :],
                                    in_=wk[:, cch * 8:(cch + 1) * 8, :])
            for cch in range(2):
                nc.sync.dma_start(out=xt0[:, cch * 8:(cch + 1) * 8, :],
                                  in_=xt[:, 0, cch * 8:(cch + 1) * 8, :])
            for cch in range(2, 4):
                nc.scalar.dma_start(out=xt0[:, cch * 8:(cch + 1) * 8, :],
                                    in_=xt[:, 0, cch * 8:(cch + 1) * 8, :])
            nc.sync.dma_start(out=wq_sb[:, 0, :, :], in_=wq[:, 0, :, :])
            nc.sync.dma_start(out=ball_sb, in_=ball[:, :])
            nc.sync.dma_start(out=wv_sb, in_=wv[:, :, :])
            for f in range(1, HPC):
                nc.sync.dma_start(out=wq_sb[:, f, :, :], in_=wq[:, f, :, :])
            make_identity(nc, ident)
            nc.vector.memset(ones_mat, 1.0)

            # PE pre-warm: run throwaway matmuls while the first weight DMAs
            # land, so the tensor engine's p-state clock is fully ramped when
            # real work starts. Output is drained to a scratch tile, unused.
            warm_ps = pc.tile([128, 128], F32, tag="cgrp", name="warmps")
            for w in range(24):
                nc.tensor.matmul(warm_ps, lhsT=ones_mat, rhs=ones_mat,
                                 start=(w == 0), stop=(w == 23))
            warm_sb = accp.tile([128, 128], BF16, tag="vstage", name="warmsb")
            nc.vector.tensor_copy(out=warm_sb, in_=warm_ps)

            # =========== Phase A: QKV projections ===========
            def emit_A_group(tt, g, xt_t):
                """One of the six matmul groups of A-tile tt: g 0..3 = q-head
                features, 4 = k, 5 = v (+ V transposes)."""
                t0 = tt * TT
                with nc.named_scope("phaseA"):
                    if g < HPC:
                        f = g
                        ps = pm.tile([128, TT], F32, tag="psm",
                                     name=f"psq{tt}_{f}")
                        for kt in range(32):
                            nc.tensor.matmul(
                                ps, lhsT=wq_sb[:, f, kt, :],
                                rhs=xt_t[:, kt, :],
                                start=(kt == 0), stop=(kt == 31))
                        nc.scalar.activation(
                            out=q_sb[:, f, t0:t0 + TT], in_=ps, func=Id,
                            bias=ball_sb[:, f:f + 1], scale=1.0)
                    elif g == HPC:
                        ps = pm.tile([128, TT], F32, tag="psm",
                                     name=f"psk{tt}")
                        for kt in range(32):
                            nc.tensor.matmul(ps, lhsT=wk_sb[:, kt, :],
                                             rhs=xt_t[:, kt, :],
                                             start=(kt == 0), stop=(kt == 31))
                        nc.scalar.activation(
                            out=k_sb[:, t0:t0 + TT], in_=ps,
                            func=Id, bias=bk_ap, scale=1.0)
                    else:
                        ps = pm.tile([128, TT], F32, tag="psm",
                                     name=f"psv{tt}")
                        for kt in range(32):
                            nc.tensor.matmul(ps, lhsT=wv_sb[:, kt, :],
                                             rhs=xt_t[:, kt, :],
                                             start=(kt == 0), stop=(kt == 31))
                        vstage = accp.tile([128, TT], BF16, tag="vstage",
                                           name=f"vs{tt}")
                        nc.scalar.activation(out=vstage, in_=ps, func=Id,
                                             bias=bv_ap, scale=1.0)
                        for j in range(TT // 128):
                            trp = pc.tile([128, 128], BF16, tag="cgrp",
                                          name=f"tr{tt}_{j}")
                            nc.tensor.transpose(
                                trp, vstage[:, j * 128:(j + 1) * 128], ident)
                            nc.vector.tensor_copy(
                                out=v_sb[:, tt * (TT // 128) + j, :], in_=trp)

            # `interleave`: a list of thunks (warmup attention score+exp
            # emissions) sprinkled between the six matmul groups of this tile
            def emit_A(tt, interleave=()):
                il = list(interleave)
                ipos = [0]

                def sprinkle(k):
                    for _ in range(k):
                        if ipos[0] < len(il):
                            il[ipos[0]]()
                            ipos[0] += 1

                if tt == 0:
                    xt_t = xt0
                else:
                    xt_t = stream.tile([128, 32, TT], BF16, tag="s16k",
                                       name=f"xt{tt}")
                    nc.scalar.dma_start(out=xt_t, in_=xt[:, tt, :, :])
                # Tile 0 matches the DMA arrival order: wk, wq_f0, wv, wq_f1+
                order = (4, 0, 5, 1, 2, 3) if tt == 0 else (0, 1, 2, 3, 4, 5)
                for g in order:
                    emit_A_group(tt, g, xt_t)
                    if g < HPC:
                        sprinkle(3)
                    elif g == HPC:
                        sprinkle(2)
                    else:
                        sprinkle(len(il))

            # =========== Phase B: attention, software-pipelined ===========
            # step key = (b, q4, h); 32 steps.
            state = {}

            def open_step(key):
                b, q4, h = key
                state[key] = {"p": stream.tile(
                    [128, 16, 512], BF16, tag="s16k",
                    name=f"p{b}_{q4}_{h}")}

            def emit_S1_mm(key, kt):
                """Score matmul for kt; on odd kt also the pair exp."""
                b, q4, h = key
                st = state[key]
                with nc.named_scope("phaseB"):
                    koff = b * S + kt * 128
                    qoff = b * S + q4 * 512
                    if kt % 2 == 0:
                        st["pair"] = ppair.tile(
                            [128, 2, 512], F32, tag="pair",
                            name=f"pss{b}_{q4}_{h}_{kt}")
                    nc.tensor.matmul(st["pair"][:, kt % 2, :],
                                     lhsT=k_sb[:, koff:koff + 128],
                                     rhs=q_sb[:, h, qoff:qoff + 512],
                                     start=True, stop=True)
                    if kt % 2 == 1:
                        nc.scalar.activation(
                            out=st["p"][:, kt - 1:kt + 1, :],
                            in_=st["pair"], func=Exp, scale=SCALE)

            def emit_S2(key):
                b, q4, h = key
                st = state[key]
                p_t = st["p"]
                with nc.named_scope("phaseB"):
                    t8 = treep.tile([128, 8, 512], BF16, tag="t8",
                                    name=f"t8_{b}_{q4}_{h}")
                    nc.vector.tensor_add(t8, p_t[:, 0:8, :], p_t[:, 8:16, :])
                    t4 = treep.tile([128, 4, 512], BF16, tag="t4",
                                    name=f"t4_{b}_{q4}_{h}")
                    nc.vector.tensor_add(t4, t8[:, 0:4, :], t8[:, 4:8, :])
                    t2 = treep.tile([128, 2, 512], BF16, tag="t2",
                                    name=f"t2_{b}_{q4}_{h}")
                    nc.vector.tensor_add(t2, t4[:, 0:2, :], t4[:, 2:4, :])
                    acc = accp.tile([128, 512], BF16, tag="acc",
                                    name=f"acc{b}_{q4}_{h}")
                    nc.vector.tensor_add(acc, t2[:, 0, :], t2[:, 1, :])
                    st["acc"] = acc

            def emit_S3(key):
                # One ones-matrix matmul sums acc over the 128 k-partitions
                # and broadcasts the result to all 128 partitions; then
                # 1/den = Exp(-Ln(den)) on the scalar engine (same ACT table
                # set as the softmax Exp, so no table reloads).
                b, q4, h = key
                st = state[key]
                with nc.named_scope("phaseB"):
                    den_ps = pc.tile([128, 512], F32, tag="cgrp",
                                     name=f"dn{b}_{q4}_{h}")
                    nc.tensor.matmul(den_ps, lhsT=ones_mat, rhs=st["acc"],
                                     start=True, stop=True)
                    lden = ldenp.tile([128, 512], F32, tag="lden",
                                      name=f"ld{b}_{q4}_{h}")
                    nc.scalar.activation(out=lden, in_=den_ps, func=Ln)
                    rb = rbp.tile([128, 512], BF16, tag="rb",
                                  name=f"rb{b}_{q4}_{h}")
                    nc.scalar.activation(out=rb, in_=lden, func=Exp,
                                         scale=-1.0)
                    st["rb"] = rb

            def emit_S4_av(key, kt0, kt1):
                b, q4, h = key
                st = state[key]
                with nc.named_scope("phaseB"):
                    if kt0 == 0:
                        st["psm"] = pm.tile([128, 512], F32, tag="psm",
                                            name=f"psm{b}_{q4}_{h}")
                    for kt in range(kt0, kt1):
                        nc.tensor.matmul(st["psm"],
                                         lhsT=v_sb[:, b * 16 + kt, :],
                                         rhs=st["p"][:, kt, :],
                                         start=(kt == 0), stop=(kt == 15))

            def emit_S4_mul(key):
                b, q4, h = key
                st = state[key]
                with nc.named_scope("phaseB"):
                    mid = midp.tile([128, 512], BF16, tag="mid",
                                    name=f"mid{b}_{q4}_{h}")
                    nc.vector.tensor_mul(out=mid, in0=st["psm"], in1=st["rb"])
                    st["mid"] = mid

            # =========== Phase C: one output n-group ===========
            # group = (tt32, n): out tokens [tt32*128,+128), cols [n*512,+512)
            ndrain = [0]

            def emit_C_group(tt32, n, drain, pool="pc", dma_eng="sync"):
                b = tt32 // 16
                q4 = (tt32 % 16) // 4
                j = tt32 % 4
                with nc.named_scope("phaseC"):
                    if pool == "pm":
                        pso = pm.tile([128, 512], F32, tag="psm",
                                      name=f"pso{tt32}_{n}")
                    elif pool == "pair":
                        pso = ppair.tile([128, 2, 512], F32, tag="pair",
                                         name=f"pso{tt32}_{n}")[:, 0, :]
                    else:
                        pso = pc.tile([128, 512], F32, tag="cgrp",
                                      name=f"pso{tt32}_{n}")
                    for h in range(HPC):
                        mid = state[(b, q4, h)]["mid"]
                        nc.tensor.matmul(
                            pso, lhsT=mid[:, j * 128:(j + 1) * 128],
                            rhs=wo_sb[:, h, n * 512:(n + 1) * 512],
                            start=(h == 0), stop=(h == HPC - 1))
                    ot = outp.tile([128, 512], BF16, tag="outstage",
                                   name=f"ot{tt32}_{n}")
                    # ACT carries the pair-exps + Ln/Exp chain; give it only
                    # 1 in 4 of the C drains, the rest go to the DVE
                    if drain == "alt":
                        drain = "s" if ndrain[0] % 4 == 3 else "v"
                    elif drain == "alt2":
                        drain = "s" if ndrain[0] % 2 == 1 else "v"
                    if drain == "v":
                        nc.vector.tensor_copy(out=ot, in_=pso)
                    else:
                        nc.scalar.copy(out=ot, in_=pso)
                    ndrain[0] += 1
                    eng = nc.sync if dma_eng == "sync" else nc.scalar
                    eng.dma_start(
                        out=out[tt32 * 128:(tt32 + 1) * 128,
                                n * 512:(n + 1) * 512],
                        in_=ot)

            # ---------------- program order ----------------
            steps = [(b, q4, h)
                     for b in range(B) for q4 in range(4) for h in range(HPC)]
            nsteps = len(steps)

            # A tiles 0..13 plain; warmup S1+exp of B-steps 0/1 interleaved
            # into tiles 14/15 (their q/k inputs are complete after tile 13)
            for tt in range(NTT - 2):
                emit_A(tt)
            for w, tt in ((0, NTT - 2), (1, NTT - 1)):
                open_step(steps[w])
                emit_A(tt, interleave=[
                    (lambda key=steps[w], kt=kt: emit_S1_mm(key, kt))
                    for kt in range(16)])
            emit_S2(steps[0])

            # wo shares the wq slot; load once phase A's last wq read retires
            wo_sb = persist.tile([128, HPC, HID], BF16, tag="wbig")
            nc.sync.dma_start(out=wo_sb, in_=wo[:, :, :])

            # C n-group queue: groups for q4-chunk become ready once all 4
            # heads' muls are emitted (mul for step s lands in slot s+2).
            cqueue = []
            cq_pos = [0]

            def push_ready_groups(slot):
                # step index s completes its mul in slot s+2; a q4 chunk
                # (steps 4k..4k+3) unlocks its 4 token-tiles after slot 4k+5
                while True:
                    k = len(cqueue) // 32  # q4 chunks fully pushed so far
                    if k >= 8 or 4 * k + 5 > slot:
                        break
                    b, q4 = steps[4 * k][0], steps[4 * k][1]
                    for j in range(4):
                        tt32 = b * 16 + q4 * 4 + j
                        for n in range(8):
                            cqueue.append((tt32, n))

            def emit_C_quota(quota, drain="alt"):
                e = 0
                while e < quota and cq_pos[0] < len(cqueue):
                    emit_C_group(*cqueue[cq_pos[0]], drain=drain)
                    cq_pos[0] += 1
                    e += 1

            # main slot loop; slots 0/1 were absorbed into the A-tail prologue
            for i in range(2, nsteps + 2):
                s1 = steps[i] if i < nsteps else None
                s2 = steps[i - 1] if 1 <= i - 1 < nsteps else None
                s34 = steps[i - 2] if 2 <= i else None

                if s1 is not None:
                    open_step(s1)
                if s34 is not None:
                    # first AV quarter up front: psm allocates while its ring
                    # is empty, so it never stalls behind a full ring
                    emit_S4_av(s34, 0, 4)
                if s2 is not None:
                    emit_S2(s2)
                # interleave: 4-score bursts, AV quarters, C groups; the
                # denominator chain (S3) goes at kt==2 so the scalar engine
                # starts the slot with the first pair exp, not with Ln
                for kt in range(16):
                    if s1 is not None:
                        emit_S1_mm(s1, kt)
                    if kt == 2 and s34 is not None:
                        emit_S3(s34)
                    if kt % 4 == 1 and kt > 1 and s34 is not None:
                        emit_S4_av(s34, kt - 1, kt + 3)
                    if kt % 4 == 3:
                        emit_C_quota(2)
                if s34 is not None:
                    emit_S4_mul(s34)
                push_ready_groups(i)
                # top-up C toward a uniform drain rate over the slot budget.
                # In the last slots the scalar engine is winding down its
                # exps, so split drains 50/50 to keep the DVE from backing up
                # (a late S4_mul stalls the next psm allocation on the PE).
                want = (len(cqueue) * (i + 1)) // (nsteps + 2)
                cap = 16 if i >= nsteps - 4 else 12
                dr = "alt2" if i >= nsteps - 6 else "alt"
                emit_C_quota(max(0, min(want - cq_pos[0], cap)), drain=dr)

            # flush remaining output groups: the pair/psm PSUM rings are idle
            # now, so cycle all three pools (6 banks) and split drains evenly
            # between the scalar and vector engines to avoid ring stalls
            push_ready_groups(1000)
            fl = 0
            while cq_pos[0] < len(cqueue):
                emit_C_group(*cqueue[cq_pos[0]],
                             drain=("v" if fl % 2 == 0 else "s"),
                             pool=("pc", "pm", "pair")[fl % 3])
                cq_pos[0] += 1
                fl += 1

    return nc


_cached_nc = None


def _get_program():
    global _cached_nc
    if _cached_nc is None:
        _cached_nc = _build_program()
        _legalize_single_wait(_cached_nc)
    return _cached_nc


def kernel(X, Wq, bq, Wk, bk, Wv, bv, Wo, bo):
    global LAST_RESULT
    X = np.asarray(X, np.float32)
    Wq = np.asarray(Wq, np.float32)
    Wk = np.asarray(Wk, np.float32)
    Wv = np.asarray(Wv, np.float32)
    Wo = np.asarray(Wo, np.float32)
    bq = np.asarray(bq, np.float32)
    bk = np.asarray(bk, np.float32)
    bv = np.asarray(bv, np.float32)
    bo = np.asarray(bo, np.float32)

    XT = np.ascontiguousarray(X.reshape(T, HID).T)          # [HID, T]
    xt_host = np.ascontiguousarray(
        XT.reshape(32, 128, NTT, TT).transpose(1, 2, 0, 3)).astype(npbf16)

    in_maps = []
    for c in range(NCORES):
        wq_c = Wq[:, c * FPC:(c + 1) * FPC]
        wk_c = Wk[:, c * D:(c + 1) * D]
        wv_c = Wv[:, c * D:(c + 1) * D]
        wo_c = Wo[c * FPC:(c + 1) * FPC, :]
        ball_c = np.concatenate([
            bq[c * FPC:(c + 1) * FPC].reshape(HPC, 128).T,
            bk[c * D:(c + 1) * D].reshape(D, 1),
            bv[c * D:(c + 1) * D].reshape(D, 1),
        ], axis=1)
        in_maps.append({
            "xt": xt_host,
            "wq": np.ascontiguousarray(
                wq_c.reshape(32, 128, HPC, D).transpose(1, 2, 0, 3),
                ).astype(npbf16),
            "wk": np.ascontiguousarray(
                wk_c.reshape(32, 128, D).transpose(1, 0, 2)).astype(npbf16),
            "wv": np.ascontiguousarray(
                wv_c.reshape(32, 128, D).transpose(1, 0, 2)).astype(npbf16),
            "wo": np.ascontiguousarray(
                wo_c.reshape(HPC, 128, HID).transpose(1, 0, 2)).astype(npbf16),
            "ball": np.ascontiguousarray(ball_c).astype(np.float32),
        })

    nc = _get_program()
    res = run_bass_kernel_spmd(
        nc, in_maps, list(range(NCORES)),
        tmpdir=os.environ.get("BASS_TMPDIR") or None)
    LAST_RESULT = res

    acc = np.zeros((T, HID), np.float32)
    for c in range(NCORES):
        acc += np.asarray(res.results[c]["out"], np.float32)
    return (acc + bo[None, :]).reshape(B, S, HID).astype(np.float32)


# revision 35
# speedup vs baseline: 1.0080x; 1.0021x over previous
"""GQA kernel for Trainium2, tensor-parallel over 8 NeuronCores.

Problem: X[2,2048,4096] -> GQA(H=32 heads, G=8 kv-groups, D=128) -> out[2,2048,4096].
Sharding: core c owns query heads 4c..4c+3 (512 q-features) and KV group c
(128 kv-features), plus the matching Wo row-slice. Each core computes a full
[4096,4096] partial of the output projection; the host sums the 8 partials
(bf16) and adds bo.

Device layouts (feature-on-partition so no activations need transposing
except V, which gets 32 PE transposes):
  Q_T [128d, 4h, 4096tok]  K_T [128d, 4096tok]  V [128tok, 32tt, 128d]
  scores_T[k,q] = K_tile.T @ Q_tile    (d contracts on partitions)
  Score PSUM comes in 2-bank pairs so ONE exp covers [128,1024] (halves the
  scalar-engine instruction count). Softmax denominator: bf16 DVE tree over
  the 16 kt tiles, then ONE ones-matrix matmul sums the 128 k-partitions AND
  broadcasts to all 128 output partitions; 1/den is computed as Exp(-Ln(den))
  on the scalar engine (both functions live in the same ACT table set, and
  the DVE reciprocal instruction is ~6.5ns/elem — far slower).
  mid_T[d,q] = sum_kt V_tile.T @ P_tile, scaled by the broadcast 1/denom
  out[tok,hid] = sum_h mid_T_h.T @ Wo_h  (accumulated over the 4 heads in PSUM)

Schedule: phase A (QKV proj) runs 16 token-tiles at ~100% PE, with K/V
projections first on tile 0 so compute starts as soon as the small wk DMA
lands (xt streams on the scalar-engine DMA queue in parallel with weights on
the sync queue). Phase B runs 32 softmax steps (b, q4, h) in a 3-deep
software pipeline with the phase-C output-projection n-groups interleaved
into every step as PE filler.
"""

import os

import ml_dtypes
import numpy as np

import concourse.bass as bass
import concourse.mybir as mybir
import concourse.tile as tile
from concourse.bass_utils import run_bass_kernel_spmd
from concourse.masks import make_identity

B, S, HID = 2, 2048, 4096
H, G, D = 32, 8, 128
T = B * S            # 4096 tokens
NCORES = 8
HPC = H // NCORES    # 4 heads per core
FPC = HPC * D        # 512 q-features per core
TT = 256             # phase-A token tile
NTT = T // TT        # 16
SCALE = 1.0 / float(np.sqrt(D))

BF16 = mybir.dt.bfloat16
F32 = mybir.dt.float32
npbf16 = ml_dtypes.bfloat16

LAST_RESULT = None  # test.py reads exec_time_ns / trace path from here


def _legalize_single_wait(nc):
    """This container's walrus accepts at most ONE sync-wait per instruction;
    Tile emits several. Split surplus waits onto EVSEM carrier instructions
    inserted just before the offender on the same engine."""
    n_carrier = 0
    for f in nc.m.functions:
        for blk in f.blocks:
            insts = blk.instructions
            out = []
            changed = False
            for ins in insts:
                si = ins.sync_info
                if si is not None and len(si.on_wait) > 1:
                    waits = list(si.on_wait)
                    ups = list(si.on_update)
                    for w in waits[:-1]:
                        n_carrier += 1
                        ev = mybir.InstEventSemaphore(
                            name=f"I-waitfix-{n_carrier}")
                        ev.engine = ins.engine
                        ev.sync_info = mybir.SyncInfo(on_wait=[w], on_update=[])
                        out.append(ev)
                    ins.sync_info = mybir.SyncInfo(
                        on_wait=[waits[-1]], on_update=ups)
                    changed = True
                out.append(ins)
            if changed:
                blk.instructions = out
    return n_carrier


def _build_program():
    nc = bass.Bass()

    xt = nc.dram_tensor("xt", [128, NTT, 32, TT], BF16, kind="ExternalInput")
    # wq is f-major: [128, HPC, 32kt, 128] so one Q feature-group's weights
    # are a single contiguous 1MB DMA with 8KB partition rows
    wq = nc.dram_tensor("wq", [128, HPC, 32, D], BF16, kind="ExternalInput")
    wk = nc.dram_tensor("wk", [128, 32, D], BF16, kind="ExternalInput")
    wv = nc.dram_tensor("wv", [128, 32, D], BF16, kind="ExternalInput")
    wo = nc.dram_tensor("wo", [128, HPC, HID], BF16, kind="ExternalInput")
    # bq (HPC cols) | bk | bv packed in one tensor -> one small DMA
    ball = nc.dram_tensor("ball", [128, HPC + 2], F32, kind="ExternalInput")
    out = nc.dram_tensor("out", [T, HID], BF16, kind="ExternalOutput")

    Id = mybir.ActivationFunctionType.Identity
    Exp = mybir.ActivationFunctionType.Exp
    Ln = mybir.ActivationFunctionType.Ln

    with nc.allow_low_precision("bf16 softmax denominator / scale factors"), \
         tile.TileContext(nc) as tc:
        with (
            tc.tile_pool(name="persist", bufs=1) as persist,
            tc.tile_pool(name="stream", bufs=4) as stream,
            tc.tile_pool(name="tree", bufs=1) as treep,
            tc.tile_pool(name="accp", bufs=2) as accp,
            tc.tile_pool(name="ldenp", bufs=2) as ldenp,
            tc.tile_pool(name="rbp", bufs=2) as rbp,
            tc.tile_pool(name="midp", bufs=14) as midp,
            tc.tile_pool(name="outp", bufs=4) as outp,
            tc.tile_pool(name="ppair", bufs=2, space="PSUM") as ppair,
            tc.tile_pool(name="pm", bufs=2, space="PSUM") as pm,
            tc.tile_pool(name="pc", bufs=2, space="PSUM") as pc,
        ):
            # ---- persistent SBUF tensors ----
            wq_sb = persist.tile([128, HPC, 32, D], BF16, tag="wbig")
            wk_sb = persist.tile([128, 32, D], BF16, tag="wk")
            wv_sb = persist.tile([128, 32, D], BF16, tag="wv")
            q_sb = persist.tile([128, HPC, T], BF16, tag="q")
            k_sb = persist.tile([128, T], BF16, tag="k")
            v_sb = persist.tile([128, 32, D], BF16, tag="v")
            ball_sb = persist.tile([128, HPC + 2], F32, tag="ball")
            ident = persist.tile([128, 128], BF16, tag="ident")
            ones_mat = persist.tile([128, 128], BF16, tag="onesm")
            bk_ap = ball_sb[:, HPC:HPC + 1]
            bv_ap = ball_sb[:, HPC + 1:HPC + 2]

            # Startup DMAs. Weights go on the Sync HWDGE queue; xt token
            # tiles stream on the Scalar HWDGE queue so the two transfer in
            # parallel. Tile 0's group order is K, Q0, V, Q1..Q3 to match
            # the DMA arrival order below.
            # The K->Q0 startup chain rides the Sync queue: xt0's first half
            # and wq_f0. The small wk chunks ride the Scalar queue alone, so
            # neither queue blocks the other's critical bytes.
            xt0 = stream.tile([128, 32, TT], BF16, tag="s16k", name="xt0")
            for cch in range(4):
                nc.scalar.dma_start(out=wk_sb[:, cch * 8:(cch + 1) * 8, :],
                                    in_=wk[:, cch * 8:(cch + 1) * 8, :])
            for cch in range(2):
                nc.sync.dma_start(out=xt0[:, cch * 8:(cch + 1) * 8, :],
                                  in_=xt[:, 0, cch * 8:(cch + 1) * 8, :])
            for cch in range(2, 4):
                nc.scalar.dma_start(out=xt0[:, cch * 8:(cch + 1) * 8, :],
                                    in_=xt[:, 0, cch * 8:(cch + 1) * 8, :])
            nc.sync.dma_start(out=wq_sb[:, 0, :, :], in_=wq[:, 0, :, :])
            nc.sync.dma_start(out=ball_sb, in_=ball[:, :])
            nc.sync.dma_start(out=wv_sb, in_=wv[:, :, :])
            for f in range(1, HPC):
                nc.sync.dma_start(out=wq_sb[:, f, :, :], in_=wq[:, f, :, :])
            make_identity(nc, ident)
            nc.vector.memset(ones_mat, 1.0)

            # PE pre-warm: run throwaway matmuls while the first weight DMAs
            # land, so the tensor engine's p-state clock is fully ramped when
            # real work starts. Output is drained to a scratch tile, unused.
            warm_ps = pc.tile([128, 128], F32, tag="cgrp", name="warmps")
            for w in range(24):
                nc.tensor.matmul(warm_ps, lhsT=ones_mat, rhs=ones_mat,
                                 start=(w == 0), stop=(w == 23))
            warm_sb = accp.tile([128, 128], BF16, tag="vstage", name="warmsb")
            nc.vector.tensor_copy(out=warm_sb, in_=warm_ps)

            # =========== Phase A: QKV projections ===========
            def emit_A_group(tt, g, xt_t):
                """One of the six matmul groups of A-tile tt: g 0..3 = q-head
                features, 4 = k, 5 = v (+ V transposes)."""
                t0 = tt * TT
                with nc.named_scope("phaseA"):
                    if g < HPC:
                        f = g
                        ps = pm.tile([128, TT], F32, tag="psm",
                                     name=f"psq{tt}_{f}")
                        for kt in range(32):
                            nc.tensor.matmul(
                                ps, lhsT=wq_sb[:, f, kt, :],
                                rhs=xt_t[:, kt, :],
                                start=(kt == 0), stop=(kt == 31))
                        nc.scalar.activation(
                            out=q_sb[:, f, t0:t0 + TT], in_=ps, func=Id,
                            bias=ball_sb[:, f:f + 1], scale=1.0)
                    elif g == HPC:
                        ps = pm.tile([128, TT], F32, tag="psm",
                                     name=f"psk{tt}")
                        for kt in range(32):
                            nc.tensor.matmul(ps, lhsT=wk_sb[:, kt, :],
                                             rhs=xt_t[:, kt, :],
                                             start=(kt == 0), stop=(kt == 31))
                        nc.scalar.activation(
                            out=k_sb[:, t0:t0 + TT], in_=ps,
                            func=Id, bias=bk_ap, scale=1.0)
                    else:
                        ps = pm.tile([128, TT], F32, tag="psm",
                                     name=f"psv{tt}")
                        for kt in range(32):
                            nc.tensor.matmul(ps, lhsT=wv_sb[:, kt, :],
                                             rhs=xt_t[:, kt, :],
                                             start=(kt == 0), stop=(kt == 31))
                        vstage = accp.tile([128, TT], BF16, tag="vstage",
                                           name=f"vs{tt}")
                        nc.scalar.activation(out=vstage, in_=ps, func=Id,
                                             bias=bv_ap, scale=1.0)
                        for j in range(TT // 128):
                            trp = pc.tile([128, 128], BF16, tag="cgrp",
                                          name=f"tr{tt}_{j}")
                            nc.tensor.transpose(
                                trp, vstage[:, j * 128:(j + 1) * 128], ident)
                            # scalar engine has slack in phase A; keep the
                            # DVE free for the first softmax trees at the
                            # phase boundary
                            nc.scalar.copy(
                                out=v_sb[:, tt * (TT // 128) + j, :], in_=trp)

            # `interleave`: a list of thunks (warmup attention score+exp
            # emissions) sprinkled between the six matmul groups of this tile
            def emit_A(tt, interleave=()):
                il = list(interleave)
                ipos = [0]

                def sprinkle(k):
                    for _ in range(k):
                        if ipos[0] < len(il):
                            il[ipos[0]]()
                            ipos[0] += 1

                if tt == 0:
                    xt_t = xt0
                else:
                    xt_t = stream.tile([128, 32, TT], BF16, tag="s16k",
                                       name=f"xt{tt}")
                    nc.scalar.dma_start(out=xt_t, in_=xt[:, tt, :, :])
                # Tile 0 matches the DMA arrival order: wk, wq_f0, wv, wq_f1+
                order = (4, 0, 5, 1, 2, 3) if tt == 0 else (0, 1, 2, 3, 4, 5)
                for g in order:
                    emit_A_group(tt, g, xt_t)
                    if g < HPC:
                        sprinkle(3)
                    elif g == HPC:
                        sprinkle(2)
                    else:
                        sprinkle(len(il))

            # =========== Phase B: attention, software-pipelined ===========
            # step key = (b, q4, h); 32 steps.
            state = {}

            def open_step(key):
                b, q4, h = key
                state[key] = {"p": stream.tile(
                    [128, 16, 512], BF16, tag="s16k",
                    name=f"p{b}_{q4}_{h}")}

            def emit_S1_mm(key, kt):
                """Score matmul for kt; on odd kt also the pair exp."""
                b, q4, h = key
                st = state[key]
                with nc.named_scope("phaseB"):
                    koff = b * S + kt * 128
                    qoff = b * S + q4 * 512
                    if kt % 2 == 0:
                        st["pair"] = ppair.tile(
                            [128, 2, 512], F32, tag="pair",
                            name=f"pss{b}_{q4}_{h}_{kt}")
                    nc.tensor.matmul(st["pair"][:, kt % 2, :],
                                     lhsT=k_sb[:, koff:koff + 128],
                                     rhs=q_sb[:, h, qoff:qoff + 512],
                                     start=True, stop=True)
                    if kt % 2 == 1:
                        nc.scalar.activation(
                            out=st["p"][:, kt - 1:kt + 1, :],
                            in_=st["pair"], func=Exp, scale=SCALE)

            def emit_S2(key):
                b, q4, h = key
                st = state[key]
                p_t = st["p"]
                with nc.named_scope("phaseB"):
                    t8 = treep.tile([128, 8, 512], BF16, tag="t8",
                                    name=f"t8_{b}_{q4}_{h}")
                    nc.vector.tensor_add(t8, p_t[:, 0:8, :], p_t[:, 8:16, :])
                    t4 = treep.tile([128, 4, 512], BF16, tag="t4",
                                    name=f"t4_{b}_{q4}_{h}")
                    nc.vector.tensor_add(t4, t8[:, 0:4, :], t8[:, 4:8, :])
                    t2 = treep.tile([128, 2, 512], BF16, tag="t2",
                                    name=f"t2_{b}_{q4}_{h}")
                    nc.vector.tensor_add(t2, t4[:, 0:2, :], t4[:, 2:4, :])
                    acc = accp.tile([128, 512], BF16, tag="acc",
                                    name=f"acc{b}_{q4}_{h}")
                    nc.vector.tensor_add(acc, t2[:, 0, :], t2[:, 1, :])
                    st["acc"] = acc

            def emit_S3(key):
                # One ones-matrix matmul sums acc over the 128 k-partitions
                # and broadcasts the result to all 128 partitions; then
                # 1/den = Exp(-Ln(den)) on the scalar engine (same ACT table
                # set as the softmax Exp, so no table reloads).
                b, q4, h = key
                st = state[key]
                with nc.named_scope("phaseB"):
                    den_ps = pc.tile([128, 512], F32, tag="cgrp",
                                     name=f"dn{b}_{q4}_{h}")
                    nc.tensor.matmul(den_ps, lhsT=ones_mat, rhs=st["acc"],
                                     start=True, stop=True)
                    lden = ldenp.tile([128, 512], F32, tag="lden",
                                      name=f"ld{b}_{q4}_{h}")
                    nc.scalar.activation(out=lden, in_=den_ps, func=Ln)
                    rb = rbp.tile([128, 512], BF16, tag="rb",
                                  name=f"rb{b}_{q4}_{h}")
                    nc.scalar.activation(out=rb, in_=lden, func=Exp,
                                         scale=-1.0)
                    st["rb"] = rb

            def emit_S4_av(key, kt0, kt1):
                b, q4, h = key
                st = state[key]
                with nc.named_scope("phaseB"):
                    if kt0 == 0:
                        st["psm"] = pm.tile([128, 512], F32, tag="psm",
                                            name=f"psm{b}_{q4}_{h}")
                    for kt in range(kt0, kt1):
                        nc.tensor.matmul(st["psm"],
                                         lhsT=v_sb[:, b * 16 + kt, :],
                                         rhs=st["p"][:, kt, :],
                                         start=(kt == 0), stop=(kt == 15))

            def emit_S4_mul(key):
                b, q4, h = key
                st = state[key]
                with nc.named_scope("phaseB"):
                    mid = midp.tile([128, 512], BF16, tag="mid",
                                    name=f"mid{b}_{q4}_{h}")
                    nc.vector.tensor_mul(out=mid, in0=st["psm"], in1=st["rb"])
                    st["mid"] = mid

            # =========== Phase C: one output n-group ===========
            # group = (tt32, n): out tokens [tt32*128,+128), cols [n*512,+512)
            ndrain = [0]

            def emit_C_group(tt32, n, drain, pool="pc", dma_eng="sync"):
                b = tt32 // 16
                q4 = (tt32 % 16) // 4
                j = tt32 % 4
                with nc.named_scope("phaseC"):
                    if pool == "pm":
                        pso = pm.tile([128, 512], F32, tag="psm",
                                      name=f"pso{tt32}_{n}")
                    elif pool == "pair":
                        pso = ppair.tile([128, 2, 512], F32, tag="pair",
                                         name=f"pso{tt32}_{n}")[:, 0, :]
                    else:
                        pso = pc.tile([128, 512], F32, tag="cgrp",
                                      name=f"pso{tt32}_{n}")
                    for h in range(HPC):
                        mid = state[(b, q4, h)]["mid"]
                        nc.tensor.matmul(
                            pso, lhsT=mid[:, j * 128:(j + 1) * 128],
                            rhs=wo_sb[:, h, n * 512:(n + 1) * 512],
                            start=(h == 0), stop=(h == HPC - 1))
                    ot = outp.tile([128, 512], BF16, tag="outstage",
                                   name=f"ot{tt32}_{n}")
                    # ACT carries the pair-exps + Ln/Exp chain; give it only
                    # 1 in 4 of the C drains, the rest go to the DVE
                    if drain == "alt":
                        drain = "s" if ndrain[0] % 4 == 3 else "v"
                    elif drain == "alt2":
                        drain = "s" if ndrain[0] % 2 == 1 else "v"
                    if drain == "v":
                        nc.vector.tensor_copy(out=ot, in_=pso)
                    else:
                        nc.scalar.copy(out=ot, in_=pso)
                    ndrain[0] += 1
                    eng = nc.sync if dma_eng == "sync" else nc.scalar
                    eng.dma_start(
                        out=out[tt32 * 128:(tt32 + 1) * 128,
                                n * 512:(n + 1) * 512],
                        in_=ot)

            # ---------------- program order ----------------
            steps = [(b, q4, h)
                     for b in range(B) for q4 in range(4) for h in range(HPC)]
            nsteps = len(steps)

            # A tiles 0..13 plain; warmup S1+exp of B-steps 0/1 interleaved
            # into tiles 14/15 (their q/k inputs are complete after tile 13)
            for tt in range(NTT - 2):
                emit_A(tt)
            for w, tt in ((0, NTT - 2), (1, NTT - 1)):
                open_step(steps[w])
                emit_A(tt, interleave=[
                    (lambda key=steps[w], kt=kt: emit_S1_mm(key, kt))
                    for kt in range(16)])
            emit_S2(steps[0])

            # wo shares the wq slot; load once phase A's last wq read retires
            wo_sb = persist.tile([128, HPC, HID], BF16, tag="wbig")
            nc.sync.dma_start(out=wo_sb, in_=wo[:, :, :])

            # C n-group queue: groups for q4-chunk become ready once all 4
            # heads' muls are emitted (mul for step s lands in slot s+2).
            cqueue = []
            cq_pos = [0]

            def push_ready_groups(slot):
                # step index s completes its mul in slot s+2; a q4 chunk
                # (steps 4k..4k+3) unlocks its 4 token-tiles after slot 4k+5
                while True:
                    k = len(cqueue) // 32  # q4 chunks fully pushed so far
                    if k >= 8 or 4 * k + 5 > slot:
                        break
                    b, q4 = steps[4 * k][0], steps[4 * k][1]
                    for j in range(4):
                        tt32 = b * 16 + q4 * 4 + j
                        for n in range(8):
                            cqueue.append((tt32, n))

            def emit_C_quota(quota, drain="alt"):
                e = 0
                while e < quota and cq_pos[0] < len(cqueue):
                    emit_C_group(*cqueue[cq_pos[0]], drain=drain)
                    cq_pos[0] += 1
                    e += 1

            # main slot loop; slots 0/1 were absorbed into the A-tail prologue
            for i in range(2, nsteps + 2):
                s1 = steps[i] if i < nsteps else None
                s2 = steps[i - 1] if 1 <= i - 1 < nsteps else None
                s34 = steps[i - 2] if 2 <= i else None

                if s1 is not None:
                    open_step(s1)
                if s34 is not None:
                    # first AV quarter up front: psm allocates while its ring
                    # is empty, so it never stalls behind a full ring
                    emit_S4_av(s34, 0, 4)
                if s2 is not None:
                    emit_S2(s2)
                # interleave: 4-score bursts, AV quarters, C groups; the
                # denominator chain (S3) goes at kt==2 so the scalar engine
                # starts the slot with the first pair exp, not with Ln
                for kt in range(16):
                    if s1 is not None:
                        emit_S1_mm(s1, kt)
                    if kt == 2 and s34 is not None:
                        emit_S3(s34)
                    if kt % 4 == 1 and kt > 1 and s34 is not None:
                        emit_S4_av(s34, kt - 1, kt + 3)
                    if kt % 4 == 3:
                        emit_C_quota(2)
                if s34 is not None:
                    emit_S4_mul(s34)
                push_ready_groups(i)
                # top-up C toward a uniform drain rate over the slot budget.
                # In the last slots the scalar engine is winding down its
                # exps, so split drains 50/50 to keep the DVE from backing up
                # (a late S4_mul stalls the next psm allocation on the PE).
                want = (len(cqueue) * (i + 1)) // (nsteps + 2)
                cap = 16 if i >= nsteps - 4 else 12
                dr = "alt2" if i >= nsteps - 6 else "alt"
                emit_C_quota(max(0, min(want - cq_pos[0], cap)), drain=dr)

            # flush remaining output groups: the pair/psm PSUM rings are idle
            # now, so cycle all three pools (6 banks) and split drains evenly
            # between the scalar and vector engines to avoid ring stalls
            push_ready_groups(1000)
            fl = 0
            while cq_pos[0] < len(cqueue):
                emit_C_group(*cqueue[cq_pos[0]],
                             drain=("v" if fl % 2 == 0 else "s"),
                             pool=("pc", "pm", "pair")[fl % 3])
                cq_pos[0] += 1
                fl += 1

    return nc


_cached_nc = None


def _get_program():
    global _cached_nc
    if _cached_nc is None:
        _cached_nc = _build_program()
        _legalize_single_wait(_cached_nc)
    return _cached_nc


def kernel(X, Wq, bq, Wk, bk, Wv, bv, Wo, bo):
    global LAST_RESULT
    X = np.asarray(X, np.float32)
    Wq = np.asarray(Wq, np.float32)
    Wk = np.asarray(Wk, np.float32)
    Wv = np.asarray(Wv, np.float32)
    Wo = np.asarray(Wo, np.float32)
    bq = np.asarray(bq, np.float32)
    bk = np.asarray(bk, np.float32)
    bv = np.asarray(bv, np.float32)
    bo = np.asarray(bo, np.float32)

    XT = np.ascontiguousarray(X.reshape(T, HID).T)          # [HID, T]
    xt_host = np.ascontiguousarray(
        XT.reshape(32, 128, NTT, TT).transpose(1, 2, 0, 3)).astype(npbf16)

    in_maps = []
    for c in range(NCORES):
        wq_c = Wq[:, c * FPC:(c + 1) * FPC]
        wk_c = Wk[:, c * D:(c + 1) * D]
        wv_c = Wv[:, c * D:(c + 1) * D]
        wo_c = Wo[c * FPC:(c + 1) * FPC, :]
        ball_c = np.concatenate([
            bq[c * FPC:(c + 1) * FPC].reshape(HPC, 128).T,
            bk[c * D:(c + 1) * D].reshape(D, 1),
            bv[c * D:(c + 1) * D].reshape(D, 1),
        ], axis=1)
        in_maps.append({
            "xt": xt_host,
            "wq": np.ascontiguousarray(
                wq_c.reshape(32, 128, HPC, D).transpose(1, 2, 0, 3),
                ).astype(npbf16),
            "wk": np.ascontiguousarray(
                wk_c.reshape(32, 128, D).transpose(1, 0, 2)).astype(npbf16),
            "wv": np.ascontiguousarray(
                wv_c.reshape(32, 128, D).transpose(1, 0, 2)).astype(npbf16),
            "wo": np.ascontiguousarray(
                wo_c.reshape(HPC, 128, HID).transpose(1, 0, 2)).astype(npbf16),
            "ball": np.ascontiguousarray(ball_c).astype(np.float32),
        })

    nc = _get_program()
    res = run_bass_kernel_spmd(
        nc, in_maps, list(range(NCORES)),
        tmpdir=os.environ.get("BASS_TMPDIR") or None)
    LAST_RESULT = res

    acc = np.zeros((T, HID), np.float32)
    for c in range(NCORES):
        acc += np.asarray(res.results[c]["out"], np.float32)
    return (acc + bo[None, :]).reshape(B, S, HID).astype(np.float32)
